# revision 19
# baseline (speedup 1.0000x reference)
"""BGRL forward pass on 8 Trainium2 NeuronCores.

Math refactor (all exact in fp32):
  gcn(x, w, b) = segment_sum(vals * (x@w)[cols]) + b = (S @ x) @ w + b
where S is the [N,N] sparse matrix with S[rows[e], cols[e]] += vals[e].

So with xcat = [x | x2] (x2 = x + perb) and G = S @ xcat (one sparse pass):
  Gx = S@x, G2 = S@x2
  online_x = Gx@w_on + b_on          (only needed through the predictor)
  online_y = G2@w_on + b_on          (= embed - x2)
  target_y = Gx@w_tg + b_tg
  target_x = G2@w_tg + b_tg
  h1(z)    = (z@w_on + b_on)@l1w + l1b = G*@w_fold + b_fold
             with w_fold = w_on@l1w, b_fold = b_on@l1w + l1b

Sharding: destination nodes are sharded across the 8 cores in 128-row blocks,
bin-packed so every core gets the same per-slot chunk counts (the SPMD NEFF is
identical on all cores; only the data differs).  The segment-sum is computed on
the TensorEngine as one-hot matmuls: for each chunk of 128 edges,
  GxT_block += msg[128edges, 128feat].T @ onehot[128edges, 128dest]
which directly yields G transposed ([feat, dest]) - exactly the lhsT/rhs
layout every downstream matmul wants, so no activation transposes are needed
anywhere except the final embed store.

BatchNorm uses batch statistics over all 100k rows -> one tiny [128,4]
AllReduce; the embed epilogue overlaps it.  Per-row scalars (dots, squared
norms) are collected across partitions with shifted-ones matmuls into a
[block_slot, row] PSUM layout so the loss tail is a handful of big ops.
"""

import os
import sys

for _p in ("/opt/trn_rl_repo",):
    if _p not in sys.path:
        sys.path.insert(0, _p)

import numpy as np

import concourse.bass as bass
import concourse.bacc as bacc
import concourse.tile as tile
import concourse.mybir as mybir
from concourse import bass_utils
from concourse.bass import IndirectOffsetOnAxis

F32 = mybir.dt.float32
BF16 = mybir.dt.bfloat16
I32 = mybir.dt.int32
I16 = mybir.dt.int16
ALU = mybir.AluOpType
ACTF = mybir.ActivationFunctionType

N_CORES = 8
D = 128
BLK = 128            # dest rows per block slot
CG = 16              # chunks per indirect-gather call
BN_EPS = 1e-5
NRM_EPS = 1e-10      # keeps 1/sqrt in HW-valid range for all-zero rows


# ---------------------------------------------------------------------------
# Host-side graph preprocessing
# ---------------------------------------------------------------------------

QW = 32768           # node-quadrant width (dma_gather int16 index limit)
GAT_BUDGET = 26      # max chunks buffered per gather group (SBUF KB/partition)


def _build_structure(rows, cols, vals, n_nodes):
    """Shard destination blocks across cores with equalized chunk counts.

    Chunks are (dest-block x col-quadrant)-pure so each gather call's int16
    indices are local to one <=32768-row slice of xcat.  All structure
    (chunk counts, call sizes) is equalized across cores so one SPMD NEFF
    serves all 8; only the data arrays differ.
    """
    n_blocks = (n_nodes + BLK - 1) // BLK
    nb_slots = (n_blocks + N_CORES - 1) // N_CORES
    total_slots = nb_slots * N_CORES
    nq = (n_nodes + QW - 1) // QW

    blk_of_edge = rows // BLK
    quad_of_edge = cols // QW
    key = blk_of_edge.astype(np.int64) * nq + quad_of_edge
    counts_bq = np.bincount(key, minlength=n_blocks * nq).reshape(n_blocks, nq)
    chunks_bq = (counts_bq + BLK - 1) // BLK               # [n_blocks, nq]
    tot_chunks = chunks_bq.sum(axis=1)

    # dummy blocks fill the slot grid; sort blocks by total chunks desc and
    # deal round-robin so per-slot per-quadrant maxima are tight
    n_dummy = total_slots - n_blocks
    all_tot = np.concatenate([tot_chunks, np.zeros(n_dummy, dtype=np.int64)])
    order = np.argsort(-all_tot, kind="stable")
    slot_blocks = order.reshape(nb_slots, N_CORES)         # [slot, core]

    cnt_sq = np.zeros((nb_slots, nq), dtype=np.int64)
    for s in range(nb_slots):
        gbs = slot_blocks[s]
        real = gbs[gbs < n_blocks]
        if len(real):
            cnt_sq[s] = chunks_bq[real].max(axis=0)
    # every slot needs >=1 chunk so its PSUM tile gets written
    empty = cnt_sq.sum(axis=1) == 0
    cnt_sq[empty, 0] = 1
    cnt_slots = cnt_sq.sum(axis=1)
    nchunk = int(cnt_slots.sum())

    # group slots into gather groups bounded by the SBUF budget
    groups = []          # list of lists of slot ids
    cur, cur_n = [], 0
    for s in range(nb_slots):
        ns = int(cnt_slots[s])
        if cur and cur_n + ns > GAT_BUDGET:
            groups.append(cur)
            cur, cur_n = [], 0
        cur.append(s)
        cur_n += ns
    if cur:
        groups.append(cur)

    # chunk order: group-major; within a group: quadrant-major, then slot.
    # This makes each (group, quadrant) gather call's chunks contiguous.
    chunk_info = []      # per chunk: (slot, quadrant, within-(s,q) index)
    call_info = []       # per group: list of (q, n_chunks, chunk0, idxcol0)
    chunk_of_sq = {}     # (slot, q) -> first chunk id
    idxcols = 0
    for g_slots in groups:
        calls = []
        for q in range(nq):
            nc_q = int(sum(cnt_sq[s, q] for s in g_slots))
            if nc_q == 0:
                continue
            chunk0 = len(chunk_info)
            calls.append((q, nc_q, chunk0, idxcols))
            idxcols += nc_q * (BLK // 16)
            for s in g_slots:
                chunk_of_sq[(s, q)] = len(chunk_info)
                for i in range(int(cnt_sq[s, q])):
                    chunk_info.append((s, q, i))
        call_info.append(calls)
    assert len(chunk_info) == nchunk

    # per-slot chunk id list (quadrant-major order, matching PSUM loop)
    slot_chunks = [[] for _ in range(nb_slots)]
    for c, (s, q, i) in enumerate(chunk_info):
        slot_chunks[s].append(c)

    # sort edges by (block, quadrant)
    edge_order = np.argsort(key, kind="stable")
    edge_starts = np.zeros(n_blocks * nq + 1, dtype=np.int64)
    np.cumsum(counts_bq.reshape(-1), out=edge_starts[1:])

    idxw = np.zeros((N_CORES, BLK, idxcols), dtype=np.int16)
    relvals = np.zeros((N_CORES, BLK, 2 * nchunk), dtype=np.float32)
    mask98 = np.zeros((N_CORES, BLK, 2 * BLK), dtype=np.float32)
    rel_all = (rows % BLK).astype(np.float32)
    colq_all = (cols % QW).astype(np.int16)

    for j in range(N_CORES):
        for s in range(nb_slots):
            gb = slot_blocks[s, j]
            if gb >= n_blocks:
                continue
            valid = min(max(n_nodes - gb * BLK, 0), BLK)
            mask98[j, s, :valid] = 1.0
            mask98[j, s, BLK:BLK + valid] = 1.0
            for q in range(nq):
                ns = int(cnt_sq[s, q])
                if ns == 0:
                    continue
                e0, e1 = (edge_starts[gb * nq + q],
                          edge_starts[gb * nq + q + 1])
                eidx = edge_order[e0:e1]
                cnt = e1 - e0
                pad = ns * BLK
                ecols = np.zeros(pad, dtype=np.int16)
                erel = np.zeros(pad, dtype=np.float32)
                evals = np.zeros(pad, dtype=np.float32)
                ecols[:cnt] = colq_all[eidx]
                erel[:cnt] = rel_all[eidx]
                evals[:cnt] = vals[eidx]
                c0 = chunk_of_sq[(s, q)]
                relvals[j, :, c0:c0 + ns] = erel.reshape(ns, BLK).T
                relvals[j, :, nchunk + c0:nchunk + c0 + ns] = (
                    evals.reshape(ns, BLK).T)
                # idx table: call-local position k -> [k%16, col0 + k//16]
                # position of this (s,q) run inside its call:
                run0 = (c0 - [ci for (qq, nc, ci, ic) in call_info[
                    _group_of_slot(groups, s)] if qq == q][0]) * BLK
                icol0 = [ic for (qq, nc, ci, ic) in call_info[
                    _group_of_slot(groups, s)] if qq == q][0]
                k = run0 + np.arange(pad)
                idxw[j, k % 16, icol0 + k // 16] = ecols

    # the SWDGE Q7 cores each read their own 16-partition replica
    idxw[:, 16:, :] = np.tile(idxw[:, :16, :], (1, 7, 1))

    return dict(
        nb_slots=nb_slots,
        nchunk=nchunk,
        nq=nq,
        idxcols=idxcols,
        cnt_sq=cnt_sq,
        slot_blocks=slot_blocks,
        groups=groups,
        call_info=call_info,
        chunk_info=chunk_info,
        slot_chunks=slot_chunks,
        chunk_of_sq=chunk_of_sq,
        idxw=idxw,
        relvals=relvals,
        mask98=mask98,
    )


def _group_of_slot(groups, s):
    for gi, gs in enumerate(groups):
        if s in gs:
            return gi
    raise ValueError(s)


# ---------------------------------------------------------------------------
# Device program
# ---------------------------------------------------------------------------

def _build_program(n_nodes, st, prelu_a):
    """Build + compile the SPMD Tile kernel. Returns the compiled Bacc."""
    nc = bacc.Bacc(
        "TRN2",
        target_bir_lowering=False,
        debug=False,
        enable_asserts=False,
        num_devices=N_CORES,
    )

    nb = st["nb_slots"]
    nchunk = st["nchunk"]
    idxcols = st["idxcols"]
    groups = st["groups"]
    call_info = st["call_info"]
    slot_chunks = st["slot_chunks"]
    cnt_slots = [len(sc) for sc in slot_chunks]
    own_rows = nb * BLK

    # ---- kernel I/O ----
    xcat = nc.dram_tensor("xcat", [n_nodes, 2 * D], F32, kind="ExternalInput")
    idxw_d = nc.dram_tensor("idxw", [BLK, idxcols], I16, kind="ExternalInput")
    relvals_d = nc.dram_tensor("relvals", [BLK, 2 * nchunk], F32,
                               kind="ExternalInput")
    x2own_d = nc.dram_tensor("x2own", [own_rows, D], F32, kind="ExternalInput")
    big_d = nc.dram_tensor("bigmat", [BLK, BLK + nb - 1], F32,
                           kind="ExternalInput")
    iota_d = nc.dram_tensor("iota", [BLK, BLK], F32, kind="ExternalInput")
    eye_d = nc.dram_tensor("eye", [BLK, BLK], F32, kind="ExternalInput")
    wcat_d = nc.dram_tensor("wcat", [D, 3 * D], F32, kind="ExternalInput")
    l2w_d = nc.dram_tensor("l2w", [D, D], F32, kind="ExternalInput")
    vecs_d = nc.dram_tensor("vecs", [D, 10], F32, kind="ExternalInput")

    embed_d = nc.dram_tensor("embed_out", [own_rows, D], F32,
                             kind="ExternalOutput")
    dots_d = nc.dram_tensor("dots_out", [BLK, 4 * BLK], F32,
                            kind="ExternalOutput")
    nsqc_d = nc.dram_tensor("nsqc_out", [BLK, 2 * BLK], F32,
                            kind="ExternalOutput")

    a = float(prelu_a)
    big_c0 = nb - 1  # column of ones in bigmat; slot b uses cols [big_c0-b:+128]

    with tile.TileContext(nc) as tc:
        from contextlib import ExitStack
        outer = ExitStack()
        perm_pool = outer.enter_context(tc.tile_pool(name="perm", bufs=1))

        def T(shape, dtype, name):
            return perm_pool.tile(shape, dtype, name=name, tag=name)

        # ---------------- constants ----------------
        iota_t = T([BLK, BLK], F32, name="iota_t")
        eye_t = T([BLK, BLK], F32, name="eye_t")
        wcat_t = T([D, 3 * D], F32, name="wcat_t")
        l2w_t = T([D, D], F32, name="l2w_t")
        vecs_t = T([D, 10], F32, name="vecs_t")
        big_t = T([BLK, BLK + nb - 1], F32, name="big_t")
        nc.sync.dma_start(iota_t[:], iota_d.ap())
        nc.sync.dma_start(eye_t[:], eye_d.ap())
        nc.sync.dma_start(wcat_t[:], wcat_d.ap())
        nc.sync.dma_start(l2w_t[:], l2w_d.ap())
        nc.sync.dma_start(vecs_t[:], vecs_d.ap())
        nc.sync.dma_start(big_t[:], big_d.ap())
        b_on = vecs_t[:, 0:1]
        b_tg = vecs_t[:, 1:2]
        bn_g = vecs_t[:, 3:4]
        bn_b = vecs_t[:, 4:5]
        l2b = vecs_t[:, 5:6]
        two_bf = vecs_t[:, 6:7]
        bf_sq = vecs_t[:, 7:8]

        # ---------------- residents ----------------
        h1_res = T([BLK, nb * 2 * D], BF16, name="h1_res")
        cd_res = T([BLK, nb * 2 * D], BF16, name="cd_res")
        stats_blk = T([BLK, 4 * nb], F32, name="stats_blk")
        nsq_sb = T([BLK, 2 * BLK], F32, name="nsq_sb")

        pair = 2 if nb % 2 == 0 else 1
        x2own_r = x2own_d.ap().rearrange("(g two p) f -> g p two f",
                                         two=pair, p=BLK)
        embed_r = embed_d.ap().rearrange("(g two p) f -> g p two f",
                                         two=pair, p=BLK)
        relvals_r = relvals_d.ap().rearrange("p (two c) -> p two c", two=2)

        bn_scale = T([D, 2], F32, name="bn_scale")
        bn_shift = T([D, 2], F32, name="bn_shift")

        with ExitStack() as pre:
            meta_p = pre.enter_context(tc.tile_pool(name="meta", bufs=3))
            gat_p = pre.enter_context(tc.tile_pool(name="gat", bufs=3))
            nqmax = int(max(cnt_slots))
            oh_p = pre.enter_context(tc.tile_pool(name="oh", bufs=2))
            sq_p = pre.enter_context(tc.tile_pool(name="sqp", bufs=2))
            gsb_p = pre.enter_context(tc.tile_pool(name="gsb", bufs=2))
            eb_p = pre.enter_context(tc.tile_pool(name="ebp", bufs=2))
            x2_p = pre.enter_context(tc.tile_pool(name="x2p", bufs=2))
            emb_p = pre.enter_context(tc.tile_pool(name="embp", bufs=2))
            trash_p = pre.enter_context(tc.tile_pool(name="trash", bufs=2))
            gps_p = pre.enter_context(
                tc.tile_pool(name="gps", bufs=2, space="PSUM"))
            dns1_p = pre.enter_context(
                tc.tile_pool(name="dns1", bufs=2, space="PSUM"))
            dns2_p = pre.enter_context(
                tc.tile_pool(name="dns2", bufs=2, space="PSUM"))
            nsq_p = pre.enter_context(
                tc.tile_pool(name="nsqp", bufs=1, space="PSUM"))

            nsq_acc = nsq_p.tile([BLK, 2 * BLK], F32)

            state = {"emb": None, "x2b": None}

            def _do_slot(b, gt, rv_t, chunk0_g, nc_g):
                my_chunks = slot_chunks[b]
                nq_b = len(my_chunks)
                # one-hots for all chunks of this slot
                oh = oh_p.tile([BLK, nqmax * BLK], F32, tag="oh")
                for q, c in enumerate(my_chunks):
                    lc = c - chunk0_g
                    nc.vector.tensor_scalar(
                        oh[:, q * BLK:(q + 1) * BLK], iota_t[:],
                        rv_t[:, lc:lc + 1], rv_t[:, nc_g + lc:nc_g + lc + 1],
                        ALU.is_equal, ALU.mult)
                gp = gps_p.tile([BLK, 2 * D], F32, tag="gp")
                # GxT group then G2T group (same bank, sequential groups)
                for half in (0, 1):
                    for q, c in enumerate(my_chunks):
                        e0 = (c - chunk0_g) * 2 * D + half * D
                        nc.tensor.matmul(
                            gp[:, half * D:(half + 1) * D],
                            gt[:, e0:e0 + D], oh[:, q * BLK:(q + 1) * BLK],
                            start=(q == 0), stop=(q == nq_b - 1))

                # ---- dense phase ----
                g_sb = gsb_p.tile([BLK, 2 * D], F32, tag="gsb")
                nc.scalar.copy(g_sb[:], gp[:])
                dns1 = dns1_p.tile([BLK, 4 * D], F32, tag="dns1")
                dns2 = dns2_p.tile([BLK, 2 * D], F32, tag="dns2")
                # BT = w_on.T @ G2T
                nc.tensor.matmul(dns1[:, 0:D], wcat_t[:, 0:D],
                                 g_sb[:, D:2 * D], start=True, stop=True)
                # [CT | DT] = w_tg.T @ [GxT | G2T]
                nc.tensor.matmul(dns1[:, D:3 * D], wcat_t[:, D:2 * D],
                                 g_sb[:, 0:2 * D], start=True, stop=True)
                # h1nbxT, h1nbyT = w_fold.T @ GxT, G2T
                nc.tensor.matmul(dns1[:, 3 * D:4 * D], wcat_t[:, 2 * D:3 * D],
                                 g_sb[:, 0:D], start=True, stop=True)
                nc.tensor.matmul(dns2[:, 0:D], wcat_t[:, 2 * D:3 * D],
                                 g_sb[:, D:2 * D], start=True, stop=True)

                # h1 resident (bf16) + per-feature sums via ACT accumulator
                nc.scalar.activation(
                    h1_res[:, b * 2 * D:b * 2 * D + D], dns1[:, 3 * D:4 * D],
                    ACTF.Copy, accum_out=stats_blk[:, b:b + 1])
                nc.scalar.activation(
                    h1_res[:, b * 2 * D + D:(b + 1) * 2 * D], dns2[:, 0:D],
                    ACTF.Copy, accum_out=stats_blk[:, nb + b:nb + b + 1])
                # per-feature sumsq via ACT Square + accumulator
                tr = trash_p.tile([BLK, D], F32, tag="trash")
                nc.scalar.activation(
                    tr[:], dns1[:, 3 * D:4 * D], ACTF.Square,
                    accum_out=stats_blk[:, 2 * nb + b:2 * nb + b + 1])
                tr2 = trash_p.tile([BLK, D], F32, tag="trash")
                nc.scalar.activation(
                    tr2[:], dns2[:, 0:D], ACTF.Square,
                    accum_out=stats_blk[:, 3 * nb + b:3 * nb + b + 1])

                # targets: bias + cast to bf16 resident
                cd_slice = cd_res[:, b * 2 * D:(b + 1) * 2 * D]
                nc.scalar.activation(cd_slice, dns1[:, D:3 * D],
                                     ACTF.Identity, bias=b_tg)
                # per-row squared norms -> nsq_acc[slot b, :]
                sq = sq_p.tile([BLK, 2 * D], F32, tag="sq")
                nc.vector.tensor_mul(sq[:], cd_slice, cd_slice)
                nc.tensor.matmul(
                    nsq_acc[:, 0:2 * BLK],
                    big_t[:, big_c0 - b:big_c0 - b + BLK], sq[:],
                    start=(b == 0), stop=(b == nb - 1), skip_group_check=True)

                # embed branch: embedT = BT + b_on -> transpose -> + x2
                eb = eb_p.tile([BLK, D], F32, tag="eb")
                nc.scalar.activation(eb[:], dns1[:, 0:D], ACTF.Identity,
                                     bias=b_on)
                nc.tensor.transpose(dns2[:, D:2 * D], eb[:], eye_t[:])
                if b % pair == 0:
                    state["emb"] = emb_p.tile([BLK, pair * D], F32, tag="emb", name="embt")
                    state["x2b"] = x2_p.tile([BLK, pair * D], F32, tag="x2b", name="x2bt")
                    nc.sync.dma_start(state["x2b"][:], x2own_r[b // pair])
                half = (b % pair) * D
                nc.vector.tensor_add(state["emb"][:, half:half + D],
                                     state["x2b"][:, half:half + D],
                                     dns2[:, D:2 * D])
                if b % pair == pair - 1:
                    nc.sync.dma_start(embed_r[b // pair], state["emb"][:])

            # ---------------- phase A+B: gather, SpMM, dense ----------------
            blocks_emitted = 0
            for gi, g_slots in enumerate(groups):
                calls = call_info[gi]
                chunk0_g = calls[0][2]
                icol0_g = calls[0][3]
                nc_g = sum(c[1] for c in calls)
                cols_g = sum(c[1] * (BLK // 16) for c in calls)

                idx_t = meta_p.tile([BLK, cols_g], I16, tag="idx")
                nc.sync.dma_start(idx_t[:],
                                  idxw_d.ap()[:, icol0_g:icol0_g + cols_g])
                rv_t = meta_p.tile([BLK, 2 * nc_g], F32, tag="rv")
                nc.sync.dma_start(rv_t[:],
                                  relvals_r[:, :, chunk0_g:chunk0_g + nc_g])
                gt = gat_p.tile([BLK, nc_g * 2 * D], F32, tag="gt")
                for (q, nc_q, c0, ic) in calls:
                    off = c0 - chunk0_g
                    n_idx = nc_q * BLK
                    q_hi = min((q + 1) * QW, n_nodes)
                    src = xcat.ap()[q * QW:q_hi, :]
                    out_ap = gt[:, off * 2 * D:(off + nc_q) * 2 * D].rearrange(
                        "p (c e) -> p c e", e=2 * D)
                    idx_ap = idx_t[:, (ic - icol0_g):
                                   (ic - icol0_g) + nc_q * (BLK // 16)]
                    nc.gpsimd.dma_gather(out_ap, src, idx_ap, n_idx, n_idx,
                                         2 * D, single_packet=False)

                for b in g_slots:
                    _do_slot(b, gt, rv_t, chunk0_g, nc_g)
                    blocks_emitted += 1

            assert blocks_emitted == nb

            # ---------------- stats -> AllReduce -> BN params ----------------
            stats4 = T([BLK, 4], F32, name="stats4")
            nc.vector.tensor_reduce(
                stats4[:], stats_blk[:].rearrange("p (t b) -> p t b", t=4),
                mybir.AxisListType.X, ALU.add)

            with tc.tile_pool(name="ccdram", bufs=1, space="DRAM") as dramp:
                cc_in = dramp.tile([BLK, 4], F32)
                cc_out = dramp.tile([BLK, 4], F32)
                nc.sync.dma_start(cc_in[:], stats4[:])
                nc.gpsimd.collective_compute(
                    "AllReduce", ALU.add,
                    replica_groups=[list(range(N_CORES))],
                    ins=[cc_in.opt()], outs=[cc_out.opt()])
                stats_g = T([BLK, 4], F32, name="stats_g")
                nc.sync.dma_start(stats_g[:], cc_out[:])

            inv_n = 1.0 / float(n_nodes)
            munb = T([D, 2], F32, name="munb")
            nc.vector.tensor_scalar(munb[:], stats_g[:, 0:2], inv_n, None,
                                    ALU.mult)
            ex2 = T([D, 2], F32, name="ex2")
            nc.vector.tensor_scalar(ex2[:], stats_g[:, 2:4], inv_n, None,
                                    ALU.mult)
            mu = T([D, 2], F32, name="mu")
            nc.vector.tensor_scalar(mu[:], munb[:], vecs_t[:, 2:3], None,
                                    ALU.add)
            p1 = T([D, 2], F32, name="p1")
            nc.vector.scalar_tensor_tensor(p1[:], munb[:], two_bf, ex2[:],
                                           ALU.mult, ALU.add)
            p2 = T([D, 2], F32, name="p2")
            nc.vector.tensor_scalar(p2[:], p1[:], bf_sq, None, ALU.add)
            musq = T([D, 2], F32, name="musq")
            nc.vector.tensor_mul(musq[:], mu[:], mu[:])
            var = T([D, 2], F32, name="var")
            nc.vector.tensor_sub(var[:], p2[:], musq[:])
            sd = T([D, 2], F32, name="sd")
            nc.scalar.activation(sd[:], var[:], ACTF.Sqrt, bias=vecs_t[:, 8:9])
            rstd = T([D, 2], F32, name="rstd")
            nc.vector.reciprocal(rstd[:], sd[:])
            nc.vector.tensor_scalar(bn_scale[:], rstd[:], bn_g, None, ALU.mult)
            nmunb = T([D, 2], F32, name="nmunb")
            nc.vector.tensor_scalar(nmunb[:], munb[:], -1.0, None, ALU.mult)
            tmsc = T([D, 2], F32, name="tmsc")
            nc.vector.tensor_mul(tmsc[:], nmunb[:], bn_scale[:])
            nc.vector.tensor_scalar(bn_shift[:], tmsc[:], bn_b, None, ALU.add)

            # nsq -> SBUF (while pre-phase PSUM still alive)
            nc.vector.tensor_copy(nsq_sb[:], nsq_acc[:])

        # ---------------- tail: BN apply, predictor, cosine ----------------
        with ExitStack() as tl:
            hn_p = tl.enter_context(tc.tile_pool(name="hnp", bufs=2))
            r_p = tl.enter_context(tc.tile_pool(name="rp", bufs=2))
            hp_p = tl.enter_context(tc.tile_pool(name="hpp", bufs=2))
            pxs_p = tl.enter_context(tc.tile_pool(name="pxsp", bufs=2))
            cn_p = tl.enter_context(tc.tile_pool(name="cnp", bufs=2))
            ml_p = tl.enter_context(tc.tile_pool(name="mlp", bufs=2))
            px_p = tl.enter_context(
                tc.tile_pool(name="pxp", bufs=2, space="PSUM"))
            dots_p = tl.enter_context(
                tc.tile_pool(name="dotsp", bufs=1, space="PSUM"))

            dots_acc = dots_p.tile([BLK, 4 * BLK], F32)

            for b in range(nb):
                h1s = h1_res[:, b * 2 * D:(b + 1) * 2 * D]
                hn = hn_p.tile([BLK, 2 * D], F32, tag="hn")
                nc.vector.tensor_scalar(hn[:, 0:D], h1s[:, 0:D],
                                        bn_scale[:, 0:1], bn_shift[:, 0:1],
                                        ALU.mult, ALU.add)
                nc.vector.tensor_scalar(hn[:, D:2 * D], h1s[:, D:2 * D],
                                        bn_scale[:, 1:2], bn_shift[:, 1:2],
                                        ALU.mult, ALU.add)
                r = r_p.tile([BLK, 2 * D], F32, tag="r")
                nc.scalar.activation(r[:], hn[:], ACTF.Relu, scale=(1.0 - a))
                hp = hp_p.tile([BLK, 2 * D], F32, tag="hp")
                nc.vector.scalar_tensor_tensor(hp[:], hn[:], a, r[:],
                                               ALU.mult, ALU.add)
                px = px_p.tile([BLK, 2 * D], F32, tag="px")
                nc.tensor.matmul(px[:, 0:D], l2w_t[:], hp[:, 0:D],
                                 start=True, stop=True)
                nc.tensor.matmul(px[:, D:2 * D], l2w_t[:], hp[:, D:2 * D],
                                 start=True, stop=True)
                pxs = pxs_p.tile([BLK, 2 * D], F32, tag="pxs")
                nc.scalar.activation(pxs[:], px[:], ACTF.Identity, bias=l2b)

                cn32 = cn_p.tile([BLK, 2 * D], F32, tag="cn32")
                nc.vector.tensor_copy(cn32[:],
                                      cd_res[:, b * 2 * D:(b + 1) * 2 * D])
                ml = ml_p.tile([BLK, 4 * D], F32, tag="ml")
                nc.vector.tensor_mul(ml[:, 0:2 * D], pxs[:], cn32[:])
                nc.scalar.square(ml[:, 2 * D:4 * D], pxs[:])
                nc.tensor.matmul(
                    dots_acc[:], big_t[:, big_c0 - b:big_c0 - b + BLK], ml[:],
                    start=(b == 0), stop=(b == nb - 1), skip_group_check=True)

            # ship per-row dots / squared norms; host does the 0/0-exact
            # cosine division (reproduces reference NaN semantics)
            dsb = T([BLK, 4 * BLK], F32, name="dsb")
            nc.vector.tensor_copy(dsb[:], dots_acc[:])
            nc.sync.dma_start(dots_d.ap(), dsb[:])
            nc.sync.dma_start(nsqc_d.ap(), nsq_sb[:])

        outer.close()

    nc.compile()
    return nc


# ---------------------------------------------------------------------------
# Entry point
# ---------------------------------------------------------------------------

def kernel(x, perb, vals, w_on, b_on, w_tg, b_tg, l1w, l1b, bn_g, bn_b,
           prelu_a, l2w, l2b, rows, cols):
    x = np.asarray(x, dtype=np.float32)
    perb = np.asarray(perb, dtype=np.float32)
    vals = np.asarray(vals, dtype=np.float32)
    rows = np.asarray(rows, dtype=np.int32)
    cols = np.asarray(cols, dtype=np.int32)
    n_nodes = x.shape[0]

    st = _build_structure(rows, cols, vals, n_nodes)
    nb = st["nb_slots"]

    nc = _build_program(n_nodes, st, float(prelu_a))

    # host-side constant prep
    xcat = np.concatenate([x, x + perb], axis=1).astype(np.float32)
    w_fold = np.asarray(w_on, np.float32) @ np.asarray(l1w, np.float32)
    b_fold = (np.asarray(b_on, np.float32) @ np.asarray(l1w, np.float32)
              + np.asarray(l1b, np.float32))
    wcat = np.concatenate(
        [np.asarray(w_on, np.float32), np.asarray(w_tg, np.float32), w_fold],
        axis=1)
    vecs = np.zeros((D, 10), dtype=np.float32)
    vecs[:, 0] = b_on
    vecs[:, 1] = b_tg
    vecs[:, 2] = b_fold
    vecs[:, 3] = bn_g
    vecs[:, 4] = bn_b
    vecs[:, 5] = l2b
    vecs[:, 6] = 2.0 * b_fold
    vecs[:, 7] = b_fold * b_fold
    vecs[:, 8] = BN_EPS
    vecs[:, 9] = NRM_EPS
    big = np.zeros((BLK, BLK + nb - 1), dtype=np.float32)
    big[:, nb - 1] = 1.0
    iota = np.broadcast_to(np.arange(BLK, dtype=np.float32),
                           (BLK, BLK)).copy()
    eye = np.eye(BLK, dtype=np.float32)

    n_blocks = (n_nodes + BLK - 1) // BLK
    own_rows = nb * BLK
    in_maps = []
    for j in range(N_CORES):
        x2own = np.zeros((own_rows, D), dtype=np.float32)
        for s in range(nb):
            gb = st["slot_blocks"][s, j]
            if gb < n_blocks:
                r0 = gb * BLK
                w = min(n_nodes - r0, BLK)
                x2own[s * BLK:s * BLK + w] = xcat[r0:r0 + w, D:2 * D]
        in_maps.append({
            "xcat": xcat,
            "idxw": st["idxw"][j],
            "relvals": st["relvals"][j],
            "x2own": x2own,
            "bigmat": big,
            "iota": iota,
            "eye": eye,
            "wcat": wcat,
            "l2w": np.asarray(l2w, np.float32),
            "vecs": vecs,
        })

    res = bass_utils.run_bass_kernel_spmd(
        nc, in_maps, core_ids=list(range(N_CORES)),
        trace=bool(int(os.environ.get("KERNEL_TRACE", "0"))))
    kernel.last_results = res

    # unshard: embed scatter + exact cosine/NaN semantics for the loss
    embed = np.zeros((n_nodes, D), dtype=np.float32)
    total_cos = np.float32(0.0)
    for j in range(N_CORES):
        out = res.results[j]
        eo = out["embed_out"]
        for s in range(nb):
            gb = st["slot_blocks"][s, j]
            if gb < n_blocks:
                r0 = gb * BLK
                w = min(n_nodes - r0, BLK)
                embed[r0:r0 + w] = eo[s * BLK:s * BLK + w]
        dots = out["dots_out"].astype(np.float32)      # [slot, 4*BLK]
        nsqc = out["nsqc_out"].astype(np.float32)      # [slot, 2*BLK]
        mask = st["mask98"][j][:, 0:2 * BLK] > 0
        with np.errstate(divide="ignore", invalid="ignore"):
            cos = (dots[:, 0:2 * BLK]
                   / (np.sqrt(dots[:, 2 * BLK:4 * BLK]) * np.sqrt(nsqc)))
        # masked-off padding rows contribute nothing (even if 0/0 there)
        total_cos += np.where(mask, cos, np.float32(0.0)).sum(
            dtype=np.float32)
    loss = np.float32(4.0) - np.float32(2.0) * total_cos / np.float32(n_nodes)
    return (embed, np.float32(loss))
